# revision 7
# baseline (speedup 1.0000x reference)
"""BiRWKV layer kernel for 8 Trainium2 NeuronCores.

Strategy (data-parallel over B=8, one batch element per core):
  - All tensors live on-chip in (channel, time) layout: channels on the
    128 SBUF partitions (C=512 -> 4 blocks), time on the free dim.
  - Projections r/k/v for both directions are fp32r matmuls
    (lhsT = W block, rhs = x^T block), accumulating over the 4
    input-channel blocks into PSUM.
  - The WKV recurrence runs UNSTABILIZED (mathematically identical to
    the reference's log-sum-exp form):
        den_t = d * den_{t-1} + e^{k_t+u},  num_t = d * num_{t-1} + e^{k_t+u} v_t
        y_t   = (num_{t-1} + e^{k_t+u} v_t) / (den_{t-1} + e^{k_t+u})
    (the e^u factor cancels in the ratio, so it is folded into the exp
    bias once). Values stay comfortably inside fp32/bf16 range because
    |w|*T <= ~28 and k ~ N(0,1).
  - The scans run on the DVE via tensor_tensor_scan along the free dim;
    the backward direction uses reversed access patterns.
  - Division is exp(-ln(denom)) on the Scalar engine; activation calls
    are batched per table set (exp/tanh vs ln/exp) to avoid the 2.7us
    ACT table reload per switch.
  - sigmoid(r) = 0.5*(1+tanh(r/2)); the 0.5 is folded into W_out on the
    host, (1+tanh) is applied via one fused scalar_tensor_tensor.
  - Output projection consumes the (j2, t) activations directly as
    matmul lhsT; result (t, c) goes PSUM -> SBUF -> HBM.
"""

import numpy as np
import ml_dtypes

B, T, C = 8, 4096, 512
TT = 512          # time-tile width
NTT = T // TT     # 8 time tiles
CB = 4            # channel blocks of 128
SB = 4            # time tiles per activation-table sub-batch
CHUNK = 2         # time tiles per scan-chain buffer chunk

_CACHE = {}


def _apply_tile_patches():
    """walrus in this container rejects instructions with >1 sync wait
    ("Too many sync wait commands"). Split excess waits onto same-engine
    nop carriers, and do the same for the TileContext tail drain."""
    import concourse.tile as tile_mod
    from concourse import mybir
    from concourse.vector_clock import ScopedClock

    if getattr(tile_mod, "_wait_split_patched", False):
        return
    MAXW = 1

    _orig_add = tile_mod.TileContext._add_instruction

    def _split_add(self, inst):
        si = inst.sync_info
        if si is not None and si.on_wait and len(si.on_wait) > MAXW:
            waits = list(si.on_wait)
            k = 0
            while len(waits) > MAXW:
                chunk, waits = waits[:MAXW], waits[MAXW:]
                carrier = mybir.InstNoOp(
                    name=f"{inst.name}_wsplit{k}",
                    engine=inst.engine,
                    bass_nofuse=True,
                    sync_info=mybir.SyncInfo(on_wait=chunk, on_update=[]),
                )
                k += 1
                _orig_add(self, carrier)
            inst.sync_info = mybir.SyncInfo(
                on_wait=waits, on_update=list(si.on_update)
            )
        return _orig_add(self, inst)

    def _drain_and_barrier(self, tick_clock, wait_clock):
        drain_inst = self.nc.sync.drain()
        wait_clock.add_sem_waits(
            drain_inst.ins, ScopedClock({None: tick_clock.global_clock})
        )
        si = drain_inst.ins.sync_info
        if si is not None and si.on_wait and len(si.on_wait) > MAXW:
            waits = list(si.on_wait)
            drain_inst.ins.sync_info = mybir.SyncInfo(
                on_wait=waits[:MAXW], on_update=list(si.on_update)
            )
            rest = waits[MAXW:]
            while rest:
                chunk, rest = rest[:MAXW], rest[MAXW:]
                n = self.nc.sync.nop(nofuse=True)
                n.ins.sync_info = mybir.SyncInfo(on_wait=chunk, on_update=[])

        self.nc.all_engine_barrier()
        assert self.sems is not None
        popped = self.nc._tile_sem_poison_stack.pop()
        assert popped is self._sem_poison
        self.nc.clear_and_free_semaphores(list(self.sems.allocated().values()))
        self.nc.all_engine_barrier()

    tile_mod.TileContext._add_instruction = _split_add
    tile_mod.TileContext._drain_and_barrier = _drain_and_barrier
    tile_mod._wait_split_patched = True


def _build_nc():
    import concourse.bass as bass
    import concourse.tile as tile
    from concourse import mybir

    _apply_tile_patches()

    f32 = mybir.dt.float32
    f32r = mybir.dt.float32r
    bf16 = mybir.dt.bfloat16
    Alu = mybir.AluOpType
    Act = mybir.ActivationFunctionType

    nc = bass.Bass()

    xT = nc.dram_tensor("xT", [C, T], f32r, kind="ExternalInput")
    # projection weight pieces, pre-arranged host-side to (128, 4*512):
    # slice [:, kb*512 + cb*128 : +128] is the lhsT block (cin kb, jout cb)
    wnames = ["w_rf", "w_kf", "w_vf", "w_rb", "w_kb", "w_vb"]
    wdram = {
        n: nc.dram_tensor(n, [128, 4 * C], f32r, kind="ExternalInput")
        for n in wnames
    }
    # output weights (0.5 folded in), bf16, pre-arranged to (128, 8*512):
    # slice [:, g*512:(g+1)*512] is rhs block g of K (g 0..3 fwd, 4..7 bwd)
    wout_d = nc.dram_tensor("wout", [128, 8 * C], bf16, kind="ExternalInput")
    u_f_d = nc.dram_tensor("u_f", [C, 1], f32, kind="ExternalInput")
    u_b_d = nc.dram_tensor("u_b", [C, 1], f32, kind="ExternalInput")
    dec_f_d = nc.dram_tensor("dec_f", [C, 1], f32, kind="ExternalInput")
    dec_b_d = nc.dram_tensor("dec_b", [C, 1], f32, kind="ExternalInput")
    out_d = nc.dram_tensor("y", [T, C], f32, kind="ExternalOutput")

    with tile.TileContext(nc) as tc:
        with (
            tc.tile_pool(name="wp", bufs=1) as wp,            # proj weights
            tc.tile_pool(name="wo", bufs=1) as wo,            # out weights
            tc.tile_pool(name="cst", bufs=1) as cst,          # per-channel consts
            tc.tile_pool(name="chain", bufs=2) as chainp,     # scan chain chunks
            tc.tile_pool(name="ypf", bufs=1) as ypfp,         # fwd y_pre store
            tc.tile_pool(name="xt", bufs=2) as xtp,           # x^T tiles
            tc.tile_pool(name="wk", bufs=1) as wkp,           # misc work (explicit tags)
            tc.tile_pool(name="ps", bufs=1, space="PSUM") as psp,
        ):
            # ---- resident tensors ----
            wout = wo.tile([128, 8 * C], bf16)
            nc.sync.dma_start(wout[:], wout_d[:])
            u_t = {}
            dec_t = {}
            for cb in range(CB):
                sl = slice(cb * 128, (cb + 1) * 128)
                u_t[("f", cb)] = cst.tile([128, 1], f32, tag=f"uf{cb}", name=f"uf{cb}")
                nc.sync.dma_start(u_t[("f", cb)][:], u_f_d[sl, :])
                u_t[("b", cb)] = cst.tile([128, 1], f32, tag=f"ub{cb}", name=f"ub{cb}")
                nc.sync.dma_start(u_t[("b", cb)][:], u_b_d[sl, :])
                dec_t[("f", cb)] = cst.tile([128, 1], f32, tag=f"df{cb}", name=f"df{cb}")
                nc.sync.dma_start(dec_t[("f", cb)][:], dec_f_d[sl, :])
                dec_t[("b", cb)] = cst.tile([128, 1], f32, tag=f"db{cb}", name=f"db{cb}")
                nc.sync.dma_start(dec_t[("b", cb)][:], dec_b_d[sl, :])

            ypf = [ypfp.tile([128, T], bf16, tag=f"ypf{cb}", name=f"ypf{cb}") for cb in range(CB)]

            def run_phase(d):
                """d: 'f' or 'b'."""
                fwd = d == "f"
                # load this phase's projection weights
                wr = wp.tile([128, 4 * C], f32r, tag="wr")
                wk = wp.tile([128, 4 * C], f32r, tag="wk")
                wv = wp.tile([128, 4 * C], f32r, tag="wv")
                nc.sync.dma_start(wr[:], wdram["w_r" + d][:])
                nc.sync.dma_start(wk[:], wdram["w_k" + d][:])
                nc.sync.dma_start(wv[:], wdram["w_v" + d][:])

                tts = list(range(NTT)) if fwd else list(reversed(range(NTT)))
                # chain chunk buffers, keyed (cb, kind); rotate per CHUNK tiles
                chunks = {}

                def chain_chunk(cb, kind, q, fresh):
                    key = (cb, kind)
                    if fresh:
                        t = chainp.tile(
                            [128, CHUNK * TT + 1], bf16, tag=f"ch_{kind}{cb}",
                            name=f"ch_{kind}{cb}",
                        )
                        prev = chunks.get(key)
                        chunks[key] = (q, t)
                        # carry column: fwd col 0 <- prev col CHUNK*TT,
                        # bwd col CHUNK*TT <- prev col 0; memset at ends
                        if fwd:
                            if prev is None:
                                nc.vector.memset(t[:, 0:1], 0.0)
                            else:
                                nc.vector.tensor_copy(
                                    t[:, 0:1], prev[1][:, CHUNK * TT: CHUNK * TT + 1]
                                )
                        else:
                            if prev is None:
                                nc.vector.memset(
                                    t[:, CHUNK * TT: CHUNK * TT + 1], 0.0
                                )
                            else:
                                nc.vector.tensor_copy(
                                    t[:, CHUNK * TT: CHUNK * TT + 1], prev[1][:, 0:1]
                                )
                    return chunks[key][1]

                # sub-batches of SB time tiles for ACT table grouping
                for s0 in range(0, NTT, SB):
                    batch = tts[s0: s0 + SB]
                    partA = {}
                    # ---------- part A: proj, exp/tanh, scans, den/num ----------
                    for tt in batch:
                        t0 = tt * TT
                        q, first = divmod(tt, CHUNK)
                        if not fwd:
                            qfresh = first == CHUNK - 1  # descending order
                        else:
                            qfresh = first == 0
                        xts = []
                        for kb in range(4):
                            xt = xtp.tile([128, TT], f32r, tag=f"xt{kb}", name=f"xt{kb}")
                            nc.sync.dma_start(
                                xt[:], xT[kb * 128: (kb + 1) * 128, t0: t0 + TT]
                            )
                            xts.append(xt)
                        for cb in range(CB):
                            # build psums (k, v, r)
                            psk = psp.tile([128, TT], f32, tag="pk", bufs=2, name="psk")
                            psv = psp.tile([128, TT], f32, tag="pv", bufs=2, name="psv")
                            psr = psp.tile([128, TT], f32, tag="pr", bufs=2, name="psr")
                            for ps, w in ((psk, wk), (psv, wv), (psr, wr)):
                                for kb in range(4):
                                    nc.tensor.matmul(
                                        ps[:],
                                        w[:, kb * C + cb * 128: kb * C + cb * 128 + 128],
                                        xts[kb][:],
                                        start=(kb == 0),
                                        stop=(kb == 3),
                                    )
                            ek = wkp.tile([128, TT], bf16, tag="ek", bufs=3)
                            nc.scalar.activation(ek[:], psk[:], Act.Exp)
                            ekb = wkp.tile([128, TT], bf16, tag="ekb", bufs=3)
                            nc.scalar.activation(
                                ekb[:], psk[:], Act.Exp, bias=u_t[(d, cb)][:, 0:1]
                            )
                            th = wkp.tile([128, TT], bf16, tag="th", bufs=3)
                            nc.scalar.activation(
                                th[:], psr[:], Act.Tanh, bias=0.0, scale=0.5
                            )
                            ekv = wkp.tile([128, TT], bf16, tag="ekv", bufs=3)
                            nc.vector.tensor_mul(ekv[:], ek[:], psv[:])
                            ekbv = wkp.tile([128, TT], bf16, tag="ekbv", bufs=3)
                            nc.vector.tensor_mul(ekbv[:], ekb[:], psv[:])

                            denb = chain_chunk(cb, "den", q, qfresh)
                            numb = chain_chunk(cb, "num", q, qfresh)
                            loc = first * TT
                            decbc = dec_t[(d, cb)][:, 0:1].broadcast_to([128, TT])
                            if fwd:
                                nc.vector.tensor_tensor_scan(
                                    denb[:, 1 + loc: 1 + loc + TT],
                                    decbc,
                                    ek[:],
                                    denb[:, loc: loc + 1],
                                    Alu.mult,
                                    Alu.add,
                                )
                                nc.vector.tensor_tensor_scan(
                                    numb[:, 1 + loc: 1 + loc + TT],
                                    decbc,
                                    ekv[:],
                                    numb[:, loc: loc + 1],
                                    Alu.mult,
                                    Alu.add,
                                )
                                den_prev = denb[:, loc: loc + TT]
                                num_prev = numb[:, loc: loc + TT]
                            else:
                                nc.vector.tensor_tensor_scan(
                                    denb[:, loc: loc + TT][:, ::-1],
                                    decbc,
                                    ek[:, ::-1],
                                    denb[:, loc + TT: loc + TT + 1],
                                    Alu.mult,
                                    Alu.add,
                                )
                                nc.vector.tensor_tensor_scan(
                                    numb[:, loc: loc + TT][:, ::-1],
                                    decbc,
                                    ekv[:, ::-1],
                                    numb[:, loc + TT: loc + TT + 1],
                                    Alu.mult,
                                    Alu.add,
                                )
                                den_prev = denb[:, loc + 1: loc + 1 + TT]
                                num_prev = numb[:, loc + 1: loc + 1 + TT]

                            dnm = wkp.tile([128, TT], bf16, tag="dnm", bufs=SB * CB + 2)
                            nc.gpsimd.tensor_add(dnm[:], ekb[:], den_prev)
                            nmr = wkp.tile([128, TT], bf16, tag="nmr", bufs=3)
                            nc.gpsimd.tensor_add(nmr[:], ekbv[:], num_prev)
                            # numer2 = (th + 1) * numer  (fused, consumes th now)
                            nmr2 = wkp.tile(
                                [128, TT], bf16, tag="nmr2", bufs=SB * CB + 2
                            )
                            nc.vector.scalar_tensor_tensor(
                                nmr2[:], th[:], 1.0, nmr[:], Alu.add, Alu.mult
                            )
                            partA[(tt, cb)] = (dnm, nmr2)

                    # ---------- part B: ln/exp division, final y_pre ----------
                    for tt in batch:
                        t0 = tt * TT
                        ypb_tiles = []
                        for cb in range(CB):
                            dnm, nmr2 = partA[(tt, cb)]
                            lnb = wkp.tile([128, TT], f32, tag="lnb", bufs=3)
                            nc.scalar.activation(lnb[:], dnm[:], Act.Ln)
                            invb = wkp.tile([128, TT], bf16, tag="invb", bufs=3)
                            nc.scalar.activation(invb[:], lnb[:], Act.Exp, scale=-1.0)
                            if fwd:
                                nc.gpsimd.tensor_mul(
                                    ypf[cb][:, t0: t0 + TT], nmr2[:], invb[:]
                                )
                            else:
                                yb = wkp.tile(
                                    [128, TT], bf16, tag="ypb", bufs=CB * 2 + 2
                                )
                                nc.gpsimd.tensor_mul(yb[:], nmr2[:], invb[:])
                                ypb_tiles.append(yb)

                        # ---------- part C (bwd only): output matmul ----------
                        if not fwd:
                            for m in range(TT // 128):
                                pso = psp.tile([128, C], f32, tag="po", bufs=2, name="pso")
                                for cb in range(CB):
                                    nc.tensor.matmul(
                                        pso[:],
                                        ypf[cb][:, t0 + m * 128: t0 + (m + 1) * 128],
                                        wout[:, cb * C: (cb + 1) * C],
                                        start=(cb == 0),
                                        stop=False,
                                    )
                                for cb in range(CB):
                                    nc.tensor.matmul(
                                        pso[:],
                                        ypb_tiles[cb][:, m * 128: (m + 1) * 128],
                                        wout[:, (4 + cb) * C: (5 + cb) * C],
                                        start=False,
                                        stop=(cb == 3),
                                    )
                                osb = wkp.tile([128, C], f32, tag="osb", bufs=3)
                                if m % 2 == 0:
                                    nc.scalar.copy(osb[:], pso[:])
                                else:
                                    nc.vector.tensor_copy(osb[:], pso[:])
                                nc.sync.dma_start(
                                    out_d[t0 + m * 128: t0 + (m + 1) * 128, :], osb[:]
                                )

            run_phase("f")
            run_phase("b")

    return nc


def _host_prep(x, W_rkv, W_out, time_decay, time_first, time_decay_rev,
               time_first_rev):
    """Build the per-core input maps (all host-side numpy)."""
    bf16 = ml_dtypes.bfloat16
    f32 = np.float32

    # weight pieces: rkv j-layout is [dir(2), rkv(3), C]
    Wr = W_rkv.reshape(C, 2, 3, C)
    pieces = {
        "w_rf": Wr[:, 0, 0], "w_kf": Wr[:, 0, 1], "w_vf": Wr[:, 0, 2],
        "w_rb": Wr[:, 1, 0], "w_kb": Wr[:, 1, 1], "w_vb": Wr[:, 1, 2],
    }
    wmaps = {}
    for n, p in pieces.items():
        # (512 cin, 512 j) -> (128, 4*512) with [:, kb*512+j]
        wmaps[n] = np.ascontiguousarray(
            p.reshape(4, 128, C).transpose(1, 0, 2).reshape(128, 4 * C)
        ).astype(f32)

    # output weights with sigmoid's 0.5 folded in, bf16,
    # (1024 j2, 512 c) -> (128, 8*512) with [:, g*512+c]
    Wo = (0.5 * W_out).reshape(8, 128, C).transpose(1, 0, 2).reshape(128, 8 * C)
    wout = np.ascontiguousarray(Wo).astype(bf16)

    u_f = np.ascontiguousarray(time_first.reshape(C, 1)).astype(f32)
    u_b = np.ascontiguousarray(time_first_rev.reshape(C, 1)).astype(f32)
    dec_f = np.exp(-np.exp(time_decay.astype(np.float64))).reshape(C, 1).astype(f32)
    dec_b = np.exp(-np.exp(time_decay_rev.astype(np.float64))).reshape(C, 1).astype(f32)

    shared = dict(wout=wout, u_f=u_f, u_b=u_b, dec_f=dec_f, dec_b=dec_b, **wmaps)
    in_maps = []
    for b in range(B):
        m = dict(shared)
        m["xT"] = np.ascontiguousarray(x[b].T).astype(f32)
        in_maps.append(m)
    return in_maps


def kernel(x, W_rkv, W_out, time_decay, time_first, time_decay_rev,
           time_first_rev, _trace=False):
    from concourse.bass_utils import run_bass_kernel_spmd

    x = np.asarray(x, dtype=np.float32)
    W_rkv = np.asarray(W_rkv, dtype=np.float32)
    W_out = np.asarray(W_out, dtype=np.float32)
    time_decay = np.asarray(time_decay, dtype=np.float32)
    time_first = np.asarray(time_first, dtype=np.float32)
    time_decay_rev = np.asarray(time_decay_rev, dtype=np.float32)
    time_first_rev = np.asarray(time_first_rev, dtype=np.float32)

    if "nc" not in _CACHE:
        _CACHE["nc"] = _build_nc()
    nc = _CACHE["nc"]

    in_maps = _host_prep(x, W_rkv, W_out, time_decay, time_first,
                         time_decay_rev, time_first_rev)
    res = run_bass_kernel_spmd(
        nc, in_maps, core_ids=list(range(B)), trace=_trace
    )
    _CACHE["last_result"] = res
    out = np.stack([res.results[b]["y"].astype(np.float32) for b in range(B)])
    return out
